# revision 7
# baseline (speedup 1.0000x reference)
"""MoE (dense all-expert FFN with double-softmax routing) on 8 trn2 NeuronCores.

Expert-parallel: core c holds expert c's W1/W2/b1/b2 resident in SBUF and
computes its expert's routing-weighted contribution
    contrib_c = weight_c * mask_c * (swish(x @ W1[c] + b1[c]) @ W2[c] + b2[c])
for all 4096 tokens, written transposed as [1024, 4096].  The host gathers the
8 partial outputs and forms  sum_c(contrib_c)^T + x  (a pure 8-way reduction +
residual + layout transform; all matmuls / softmaxes / activations / masking
run on device).

Both big matmuls (and the router matmul) run in fp8(e4m3) DoubleRow mode:
the PE array contracts 256 elements per instruction (2 fp8 rows per PE),
2x the bf16 rate.  Inputs are pre-scaled into e4m3's sweet spot on the host
(x*4, W1*32, W2*64, Wr*32) and the scales are folded back out via the
activation `scale` argument (silu reads psum*1/128) and via host-prescaled
b2 (*64) / router-select (*1/64) constants.  fp8 pair layout: contraction
index k = chunk*256 + i*128 + p maps to SBUF [p, chunk, i, ...] so each
matmul's lhsT/rhs APs are [128, 2, n] views.

All device tensors live transposed ([feature, token]) so the contraction dim
is on SBUF partitions for every matmul.  Host prep is layout/dtype only
(transpose + fp8/bf16 cast + per-expert slicing).
"""

import os
import numpy as np
import ml_dtypes

B, D, E, U = 4096, 1024, 8, 4096
BT = 512              # token tile (matmul free dim; psum-bank max for f32)
NB = B // BT          # 8 token tiles
DC = D // 128         # 8 chunks of the model dim
DC2 = D // 256        # 4 fp8 double-row chunks of the model dim
UC = U // 128         # 32 chunks of the hidden dim
UG = U // 256         # 16 fp8 double-row chunks of the hidden dim
N_CORES = 8
P = 128

_BF16 = ml_dtypes.bfloat16
_E4M3 = ml_dtypes.float8_e4m3

SX = 4.0      # x scale into e4m3
SW1 = 32.0    # W1 scale
SW2 = 64.0    # W2 scale
SWR = 32.0    # Wr scale

_NC_CACHE = {}
LAST_RESULTS = None


def _build_nc():
    import concourse.mybir as mybir
    import concourse.tile as tile
    from concourse import bacc

    f32 = mybir.dt.float32
    bf16 = mybir.dt.bfloat16
    fp8 = mybir.dt.float8e4
    AF = mybir.ActivationFunctionType
    ALU = mybir.AluOpType
    DR = mybir.MatmulPerfMode.DoubleRow

    nc = bacc.Bacc("TRN2", target_bir_lowering=False, debug=False,
                   num_devices=N_CORES)

    xt = nc.dram_tensor("xt", [P, DC2 * 2 * B], fp8, kind="ExternalInput").ap()
    w1 = nc.dram_tensor("w1", [P, DC2 * 2 * U], fp8, kind="ExternalInput").ap()
    w2 = nc.dram_tensor("w2", [P, UG * 2 * D], fp8, kind="ExternalInput").ap()
    wr = nc.dram_tensor("wr", [P, DC2 * 2 * E], fp8, kind="ExternalInput").ap()
    br = nc.dram_tensor("br", [E, 1], f32, kind="ExternalInput").ap()
    b1 = nc.dram_tensor("b1", [P, UC], f32, kind="ExternalInput").ap()
    b2 = nc.dram_tensor("b2", [P, DC], f32, kind="ExternalInput").ap()
    selb = nc.dram_tensor("selb", [E, P], bf16, kind="ExternalInput").ap()
    ones8 = nc.dram_tensor("ones8", [E, 1], bf16, kind="ExternalInput").ap()
    ones18 = nc.dram_tensor("ones18", [1, E], bf16, kind="ExternalInput").ap()
    o = nc.dram_tensor("o", [D, B], f32, kind="ExternalOutput").ap()

    with tile.TileContext(nc) as tc:
        with (
            tc.tile_pool(name="wp", bufs=1) as wp,
            tc.tile_pool(name="hbp", bufs=2) as hbp,
            tc.tile_pool(name="r8p", bufs=4) as r8p,
            tc.tile_pool(name="r1p", bufs=2) as r1p,
            tc.tile_pool(name="scp", bufs=2) as scp,
            tc.tile_pool(name="ctp", bufs=4) as ctp,
            tc.tile_pool(name="ps1p", bufs=2, space="PSUM") as ps1p,
            tc.tile_pool(name="ps2p", bufs=2, space="PSUM") as ps2p,
            tc.tile_pool(name="psrp", bufs=2, space="PSUM") as psrp,
            tc.tile_pool(name="pssp", bufs=1, space="PSUM") as pssp,
        ):
            xat = wp.tile([P, DC2, 2, B], fp8)
            w1t = wp.tile([P, DC2, 2, U], fp8)
            w2t = wp.tile([P, UG, 2, D], fp8)
            wrt = wp.tile([P, DC2, 2, E], fp8)
            b1t = wp.tile([P, UC], f32)
            b2t = wp.tile([P, DC], f32)
            brt = wp.tile([E, 1], f32)
            selbt = wp.tile([E, P], bf16)
            o8t = wp.tile([E, 1], bf16)
            o18t = wp.tile([1, E], bf16)

            def emit_router(bt):
                # weights = softmax(softmax(x@Wr + br)), gate >0.1, row c
                # broadcast to 128 partitions (pre-scaled by 1/SW2 on host).
                # bf16 throughout: weights are ~0.125 with a >2.9% gate
                # margin, far above bf16 rounding.
                b0 = bt * BT
                # plain fp8 matmul (no DoubleRow): dual-fp8 ldweights requires
                # the pair-dim stride to be a multiple of 16B, but Wr's is E=8
                lg = psrp.tile([E, BT], f32, tag="rps")
                for dc in range(DC):
                    nc.tensor.matmul(
                        lg[:], wrt[:, dc // 2, dc % 2, :],
                        xat[:, dc // 2, dc % 2, b0 : b0 + BT],
                        start=(dc == 0), stop=(dc == DC - 1),
                    )
                t1 = r8p.tile([E, BT], bf16, tag="r8")
                nc.scalar.activation(t1[:], lg[:], AF.Exp,
                                     bias=brt[:, 0:1], scale=1.0 / (SX * SWR))
                s1 = psrp.tile([1, BT], f32, tag="rps")
                nc.tensor.matmul(s1[:], o8t[:], t1[:], start=True, stop=True)
                r1 = r1p.tile([1, BT], bf16, tag="r1")
                with nc.allow_low_precision(reason="router softmax; 2e-2 gate"):
                    nc.vector.reciprocal(r1[:], s1[:])
                rb1 = psrp.tile([E, BT], f32, tag="rps")
                nc.tensor.matmul(rb1[:], o18t[:], r1[:], start=True, stop=True)
                pp = r8p.tile([E, BT], bf16, tag="r8")
                nc.vector.tensor_tensor(pp[:], t1[:], rb1[:], ALU.mult)
                t2 = r8p.tile([E, BT], bf16, tag="r8")
                nc.scalar.activation(t2[:], pp[:], AF.Exp)
                s2 = psrp.tile([1, BT], f32, tag="rps")
                nc.tensor.matmul(s2[:], o8t[:], t2[:], start=True, stop=True)
                r2 = r1p.tile([1, BT], bf16, tag="r1")
                with nc.allow_low_precision(reason="router softmax; 2e-2 gate"):
                    nc.vector.reciprocal(r2[:], s2[:])
                rb2 = psrp.tile([E, BT], f32, tag="rps")
                nc.tensor.matmul(rb2[:], o18t[:], r2[:], start=True, stop=True)
                wg = r8p.tile([E, BT], bf16, tag="r8")
                nc.vector.tensor_tensor(wg[:], t2[:], rb2[:], ALU.mult)
                sc = r8p.tile([E, BT], bf16, tag="r8")
                nc.vector.scalar_tensor_tensor(
                    sc[:], wg[:], 0.1, wg[:], ALU.is_gt, ALU.mult
                )
                s128ps = pssp.tile([P, BT], f32, tag="pss")
                nc.tensor.matmul(s128ps[:], selbt[:], sc[:], start=True, stop=True)
                s128 = scp.tile([P, BT], f32, tag="s128")
                nc.vector.tensor_copy(s128[:], s128ps[:])
                return s128

            # small/const DMAs first so the bt=0 router isn't queued behind
            # the weights
            nc.sync.dma_start(out=b1t[:], in_=b1[:])
            nc.sync.dma_start(out=b2t[:], in_=b2[:])
            nc.sync.dma_start(out=brt[:], in_=br[:])
            nc.sync.dma_start(out=selbt[:], in_=selb[:])
            nc.sync.dma_start(out=o8t[:], in_=ones8[:])
            nc.sync.dma_start(out=o18t[:], in_=ones18[:])
            for k in range(DC2):
                nc.sync.dma_start(
                    out=wrt[:, k, :, :],
                    in_=wr[:, k * 2 * E : (k + 1) * 2 * E],
                )

            # x for token-tiles 0-1, then W1's first u-block, then the rest of
            # x, the rest of W1 (u-consumption order), then W2 (first needed
            # ~60us in).  xt dram layout: [p, dc2, i, b]; w1 [p, dc2, i, u];
            # w2 [p, g, i, d].
            xt_v = xt.rearrange("p (k i b) -> p k i b", k=DC2, i=2)
            w1_v = w1.rearrange("p (k i u) -> p k i u", k=DC2, i=2)
            w2_v = w2.rearrange("p (g i d) -> p g i d", g=UG, i=2)

            for k in range(DC2):
                nc.sync.dma_start(out=xat[:, k, :, 0 : 2 * BT],
                                  in_=xt_v[:, k, :, 0 : 2 * BT])
            for k in range(DC2):
                nc.sync.dma_start(out=w1t[:, k, :, 0:1024],
                                  in_=w1_v[:, k, :, 0:1024])
            for k in range(DC2):
                nc.sync.dma_start(out=xat[:, k, :, 2 * BT : B],
                                  in_=xt_v[:, k, :, 2 * BT : B])
            for ub in range(1, 4):
                for k in range(DC2):
                    nc.sync.dma_start(
                        out=w1t[:, k, :, ub * 1024 : (ub + 1) * 1024],
                        in_=w1_v[:, k, :, ub * 1024 : (ub + 1) * 1024],
                    )
            for g in range(UG):
                nc.sync.dma_start(out=w2t[:, g, :, :], in_=w2_v[:, g, :, :])

            s128_cur = emit_router(0)
            for bt in range(NB):
                b0 = bt * BT
                s128 = s128_cur

                # ---- h^T = swish((x@W1)*1/128 + b1) -> fp8, chunks on U
                hb = hbp.tile([P, UC, BT], fp8, tag="hb")
                for uc in range(UC):
                    ps1 = ps1p.tile([P, BT], f32, tag="ps1")
                    for k in range(DC2):
                        nc.tensor.matmul(
                            ps1[:],
                            w1t[:, k, :, uc * P : (uc + 1) * P],
                            xat[:, k, :, b0 : b0 + BT],
                            start=(k == 0), stop=(k == DC2 - 1), perf_mode=DR,
                        )
                    nc.scalar.activation(hb[:, uc, :], ps1[:], AF.Silu,
                                         bias=b1t[:, uc : uc + 1],
                                         scale=1.0 / (SX * SW1))

                # route the next token tile while this one computes
                if bt + 1 < NB:
                    s128_cur = emit_router(bt + 1)

                # ---- contrib^T = (h@W2 + 64*b2) * (scale/64)  -> DRAM
                for dc in range(DC):
                    ps2 = ps2p.tile([P, BT], f32, tag="ps2")
                    for g in range(UG):
                        nc.tensor.matmul(
                            ps2[:],
                            w2t[:, g, :, dc * P : (dc + 1) * P],
                            hb[:, 2 * g : 2 * g + 2, :],
                            start=(g == 0), stop=(g == UG - 1), perf_mode=DR,
                        )
                    ct = ctp.tile([P, BT], f32, tag="ct")
                    nc.vector.scalar_tensor_tensor(
                        ct[:], ps2[:], b2t[:, dc : dc + 1], s128[:],
                        ALU.add, ALU.mult,
                    )
                    nc.sync.dma_start(
                        out=o[dc * P : (dc + 1) * P, b0 : b0 + BT], in_=ct[:]
                    )

    nc.compile()
    return nc


def _get_nc():
    if "nc" not in _NC_CACHE:
        _NC_CACHE["nc"] = _build_nc()
    return _NC_CACHE["nc"]


def _pair_pack(a, n_pair_chunks, scale, out_cols):
    """[K, N] f32 -> [128, n_pair_chunks * 2 * N] e4m3 with
    row k = chunk*256 + i*128 + p  ->  [p, chunk, i, n]."""
    k, n = a.shape
    assert k == n_pair_chunks * 256 and n == out_cols
    q = (a * scale).astype(_E4M3)
    q = q.reshape(n_pair_chunks, 2, P, n).transpose(2, 0, 1, 3)
    return np.ascontiguousarray(q.reshape(P, n_pair_chunks * 2 * n))


def _prep_in_maps(inputs):
    x = np.asarray(inputs["x"], np.float32)
    Wr = np.asarray(inputs["Wr"], np.float32)
    br = np.asarray(inputs["br"], np.float32)
    W1 = np.asarray(inputs["W1"], np.float32)
    b1 = np.asarray(inputs["b1"], np.float32)
    W2 = np.asarray(inputs["W2"], np.float32)
    b2 = np.asarray(inputs["b2"], np.float32)

    xt8 = _pair_pack(np.ascontiguousarray(x.T), DC2, SX, B)
    wr8 = _pair_pack(Wr, DC2, SWR, E)
    br_c = np.ascontiguousarray(br.reshape(E, 1))
    ones8_c = np.ones((E, 1), _BF16)
    ones18_c = np.ones((1, E), _BF16)

    in_maps = []
    for c in range(N_CORES):
        sel = np.zeros((E, P), np.float32)
        sel[c, :] = 1.0 / SW2
        in_maps.append({
            "xt": xt8,
            "w1": _pair_pack(W1[c], DC2, SW1, U),
            "w2": _pair_pack(W2[c], UG, SW2, D),
            "wr": wr8,
            "br": br_c,
            "b1": np.ascontiguousarray(b1[c].reshape(UC, P).T),
            "b2": np.ascontiguousarray((b2[c] * SW2).reshape(DC, P).T),
            "selb": sel.astype(_BF16),
            "ones8": ones8_c,
            "ones18": ones18_c,
        })
    return in_maps


def kernel(**inputs):
    from concourse.bass_utils import run_bass_kernel_spmd

    global LAST_RESULTS

    in_maps = _prep_in_maps(inputs)
    nc = _get_nc()
    want_trace = bool(int(os.environ.get("KERNEL_TRACE", "0")))
    if not want_trace:
        # the NTFF-trace path needs antenv.axon_hooks, which this container
        # lacks; make sure a stray BASS_TRACE env can't route us into it
        os.environ["BASS_NEVER_TRACE"] = "1"
    res = run_bass_kernel_spmd(
        nc, in_maps, core_ids=list(range(N_CORES)), trace=want_trace,
    )
    LAST_RESULTS = res

    # host: 8-way partial-sum reduction + residual + transpose back
    acc = res.results[0]["o"].astype(np.float32, copy=True)
    for c in range(1, N_CORES):
        acc += res.results[c]["o"]
    out = acc.T + np.asarray(inputs["x"], np.float32)
    return np.ascontiguousarray(out)


# revision 8
# speedup vs baseline: 1.5654x; 1.5654x over previous
"""MoE (dense all-expert FFN with double-softmax routing) on 8 trn2 NeuronCores.

Expert-parallel: core c holds expert c's W1/W2/b1/b2 resident in SBUF and
computes its expert's routing-weighted contribution
    contrib_c = weight_c * mask_c * (swish(x @ W1[c] + b1[c]) @ W2[c] + b2[c])
for all 4096 tokens, written transposed as [1024, 4096].  The host gathers the
8 partial outputs and forms  sum_c(contrib_c)^T + x  (a pure 8-way reduction +
residual + layout transform; all matmuls / softmaxes / activations / masking
run on device).

Both big matmuls run in fp8(e4m3) DoubleRow mode: the PE array contracts 256
elements per instruction (2 fp8 rows per PE), ~2x the bf16 rate.  Inputs are
pre-scaled into e4m3's sweet spot on the host (x*4, W1*32, W2*64, Wr*32) and
the scales are folded back out via the activation `scale` argument (silu
reads psum*1/128) and via host-prescaled b2 (*64) / router-select (*1/64)
constants.

fp8 pair layout: contraction index k = chunk*256 + i*128 + p.  The dual-fp8
weight/stream paths want the i-pair dim ADJACENT in SBUF (small contiguous
stride; a 4KB pair stride measured ~14x slower), so every operand is stored
as [...][2, tile] blocks: x as [p, dc2, bt, 2, BT], W1 as [p, dc2, uc, 2,
128], W2 as [p, g, dc, 2, 128], h as [p, uc, BT] (uc-adjacent pairs).

All device tensors live transposed ([feature, token]) so the contraction dim
is on SBUF partitions for every matmul.  Host prep is layout/dtype only
(transpose + fp8/bf16 cast + per-expert slicing).
"""

import os
import numpy as np
import ml_dtypes

B, D, E, U = 4096, 1024, 8, 4096
BT = 512              # token tile (matmul free dim; psum-bank max for f32)
NB = B // BT          # 8 token tiles
DC = D // 128         # 8 chunks of the model dim
DC2 = D // 256        # 4 fp8 double-row chunks of the model dim
UC = U // 128         # 32 chunks of the hidden dim
UG = U // 256         # 16 fp8 double-row chunks of the hidden dim
N_CORES = 8
P = 128

_BF16 = ml_dtypes.bfloat16
_E4M3 = ml_dtypes.float8_e4m3

SX = 4.0      # x scale into e4m3
SW1 = 32.0    # W1 scale
SW2 = 64.0    # W2 scale
SWR = 32.0    # Wr scale

_NC_CACHE = {}
LAST_RESULTS = None


def _build_nc():
    import concourse.mybir as mybir
    import concourse.tile as tile
    from concourse import bacc

    f32 = mybir.dt.float32
    bf16 = mybir.dt.bfloat16
    fp8 = mybir.dt.float8e4
    AF = mybir.ActivationFunctionType
    ALU = mybir.AluOpType
    DR = mybir.MatmulPerfMode.DoubleRow

    nc = bacc.Bacc("TRN2", target_bir_lowering=False, debug=False,
                   num_devices=N_CORES)

    xt = nc.dram_tensor("xt", [P, DC2 * NB * 2 * BT], fp8,
                        kind="ExternalInput").ap()
    w1 = nc.dram_tensor("w1", [P, DC2 * UC * 2 * P], fp8,
                        kind="ExternalInput").ap()
    w2 = nc.dram_tensor("w2", [P, UG * DC * 2 * P], fp8,
                        kind="ExternalInput").ap()
    wr = nc.dram_tensor("wr", [P, DC2 * 2 * E], fp8, kind="ExternalInput").ap()
    br = nc.dram_tensor("br", [E, 1], f32, kind="ExternalInput").ap()
    b1 = nc.dram_tensor("b1", [P, UC], f32, kind="ExternalInput").ap()
    b2 = nc.dram_tensor("b2", [P, DC], f32, kind="ExternalInput").ap()
    selb = nc.dram_tensor("selb", [E, P], bf16, kind="ExternalInput").ap()
    ones8 = nc.dram_tensor("ones8", [E, 1], bf16, kind="ExternalInput").ap()
    ones18 = nc.dram_tensor("ones18", [1, E], bf16, kind="ExternalInput").ap()
    o = nc.dram_tensor("o", [D, B], f32, kind="ExternalOutput").ap()

    with tile.TileContext(nc) as tc:
        with (
            tc.tile_pool(name="wp", bufs=1) as wp,
            tc.tile_pool(name="hbp", bufs=2) as hbp,
            tc.tile_pool(name="r8p", bufs=4) as r8p,
            tc.tile_pool(name="r1p", bufs=2) as r1p,
            tc.tile_pool(name="scp", bufs=2) as scp,
            tc.tile_pool(name="ctp", bufs=4) as ctp,
            tc.tile_pool(name="ps1p", bufs=2, space="PSUM") as ps1p,
            tc.tile_pool(name="ps2p", bufs=2, space="PSUM") as ps2p,
            tc.tile_pool(name="psrp", bufs=2, space="PSUM") as psrp,
            tc.tile_pool(name="pssp", bufs=1, space="PSUM") as pssp,
        ):
            xat = wp.tile([P, DC2, NB, 2, BT], fp8)
            w1t = wp.tile([P, DC2, UC, 2, P], fp8)
            w2t = wp.tile([P, UG, DC, 2, P], fp8)
            wrt = wp.tile([P, DC2, 2, E], fp8)
            b1t = wp.tile([P, UC], f32)
            b2t = wp.tile([P, DC], f32)
            brt = wp.tile([E, 1], f32)
            selbt = wp.tile([E, P], bf16)
            o8t = wp.tile([E, 1], bf16)
            o18t = wp.tile([1, E], bf16)

            def emit_router(bt):
                # weights = softmax(softmax(x@Wr + br)), gate >0.1, row c
                # broadcast to 128 partitions (pre-scaled by 1/SW2 on host).
                # bf16 throughout: weights are ~0.125 with a >2.9% gate
                # margin, far above bf16 rounding.  Plain fp8 matmul (no
                # DoubleRow): dual-fp8 ldweights requires a 16B-multiple
                # pair stride, but Wr's is E=8.
                lg = psrp.tile([E, BT], f32, tag="rps")
                for dc in range(DC):
                    nc.tensor.matmul(
                        lg[:], wrt[:, dc // 2, dc % 2, :],
                        xat[:, dc // 2, bt, dc % 2, :],
                        start=(dc == 0), stop=(dc == DC - 1),
                    )
                t1 = r8p.tile([E, BT], bf16, tag="r8")
                nc.scalar.activation(t1[:], lg[:], AF.Exp,
                                     bias=brt[:, 0:1], scale=1.0 / (SX * SWR))
                s1 = psrp.tile([1, BT], f32, tag="rps")
                nc.tensor.matmul(s1[:], o8t[:], t1[:], start=True, stop=True)
                r1 = r1p.tile([1, BT], bf16, tag="r1")
                with nc.allow_low_precision(reason="router softmax; 2e-2 gate"):
                    nc.vector.reciprocal(r1[:], s1[:])
                rb1 = psrp.tile([E, BT], f32, tag="rps")
                nc.tensor.matmul(rb1[:], o18t[:], r1[:], start=True, stop=True)
                pp = r8p.tile([E, BT], bf16, tag="r8")
                nc.vector.tensor_tensor(pp[:], t1[:], rb1[:], ALU.mult)
                t2 = r8p.tile([E, BT], bf16, tag="r8")
                nc.scalar.activation(t2[:], pp[:], AF.Exp)
                s2 = psrp.tile([1, BT], f32, tag="rps")
                nc.tensor.matmul(s2[:], o8t[:], t2[:], start=True, stop=True)
                r2 = r1p.tile([1, BT], bf16, tag="r1")
                with nc.allow_low_precision(reason="router softmax; 2e-2 gate"):
                    nc.vector.reciprocal(r2[:], s2[:])
                rb2 = psrp.tile([E, BT], f32, tag="rps")
                nc.tensor.matmul(rb2[:], o18t[:], r2[:], start=True, stop=True)
                wg = r8p.tile([E, BT], bf16, tag="r8")
                nc.vector.tensor_tensor(wg[:], t2[:], rb2[:], ALU.mult)
                sc = r8p.tile([E, BT], bf16, tag="r8")
                nc.vector.scalar_tensor_tensor(
                    sc[:], wg[:], 0.1, wg[:], ALU.is_gt, ALU.mult
                )
                s128ps = pssp.tile([P, BT], f32, tag="pss")
                nc.tensor.matmul(s128ps[:], selbt[:], sc[:], start=True, stop=True)
                s128 = scp.tile([P, BT], f32, tag="s128")
                nc.vector.tensor_copy(s128[:], s128ps[:])
                return s128

            # small/const DMAs first so the bt=0 router isn't queued behind
            # the weights
            nc.sync.dma_start(out=b1t[:], in_=b1[:])
            nc.sync.dma_start(out=b2t[:], in_=b2[:])
            nc.sync.dma_start(out=brt[:], in_=br[:])
            nc.sync.dma_start(out=selbt[:], in_=selb[:])
            nc.sync.dma_start(out=o8t[:], in_=ones8[:])
            nc.sync.dma_start(out=o18t[:], in_=ones18[:])
            for k in range(DC2):
                nc.sync.dma_start(
                    out=wrt[:, k, :, :],
                    in_=wr[:, k * 2 * E : (k + 1) * 2 * E],
                )

            # x for token-tiles 0-1, then W1's first u-block, then the rest
            # of x, the rest of W1 (uc-consumption order), then W2 (first
            # needed ~60us in).  DRAM layouts match SBUF: xt [p,k,bt,i,b],
            # w1 [p,k,uc,i,m], w2 [p,g,dc,i,m].
            xt_v = xt.rearrange("p (k t i b) -> p k t i b", k=DC2, t=NB, i=2)
            w1_v = w1.rearrange("p (k u i m) -> p k u i m", k=DC2, u=UC, i=2)
            w2_v = w2.rearrange("p (g d i m) -> p g d i m", g=UG, d=DC, i=2)

            for k in range(DC2):
                nc.sync.dma_start(out=xat[:, k, 0:2, :, :],
                                  in_=xt_v[:, k, 0:2, :, :])
            for k in range(DC2):
                nc.sync.dma_start(out=w1t[:, k, 0:8, :, :],
                                  in_=w1_v[:, k, 0:8, :, :])
            for k in range(DC2):
                nc.sync.dma_start(out=xat[:, k, 2:NB, :, :],
                                  in_=xt_v[:, k, 2:NB, :, :])
            for ub in range(1, 4):
                for k in range(DC2):
                    nc.sync.dma_start(
                        out=w1t[:, k, ub * 8 : (ub + 1) * 8, :, :],
                        in_=w1_v[:, k, ub * 8 : (ub + 1) * 8, :, :],
                    )
            for g in range(UG):
                nc.sync.dma_start(out=w2t[:, g, :, :, :], in_=w2_v[:, g, :, :, :])

            s128_cur = emit_router(0)
            for bt in range(NB):
                b0 = bt * BT
                s128 = s128_cur

                # ---- h^T = swish((x@W1)*1/128 + b1) -> fp8, chunks on U
                hb = hbp.tile([P, UC, BT], fp8, tag="hb")
                for uc in range(UC):
                    ps1 = ps1p.tile([P, BT], f32, tag="ps1")
                    for k in range(DC2):
                        nc.tensor.matmul(
                            ps1[:],
                            w1t[:, k, uc, :, :],
                            xat[:, k, bt, :, :],
                            start=(k == 0), stop=(k == DC2 - 1), perf_mode=DR,
                        )
                    nc.scalar.activation(hb[:, uc, :], ps1[:], AF.Silu,
                                         bias=b1t[:, uc : uc + 1],
                                         scale=1.0 / (SX * SW1))

                # route the next token tile while this one computes
                if bt + 1 < NB:
                    s128_cur = emit_router(bt + 1)

                # ---- contrib^T = (h@W2 + 64*b2) * (scale/64)  -> DRAM
                for dc in range(DC):
                    ps2 = ps2p.tile([P, BT], f32, tag="ps2")
                    for g in range(UG):
                        nc.tensor.matmul(
                            ps2[:],
                            w2t[:, g, dc, :, :],
                            hb[:, 2 * g : 2 * g + 2, :],
                            start=(g == 0), stop=(g == UG - 1), perf_mode=DR,
                        )
                    ct = ctp.tile([P, BT], f32, tag="ct")
                    nc.vector.scalar_tensor_tensor(
                        ct[:], ps2[:], b2t[:, dc : dc + 1], s128[:],
                        ALU.add, ALU.mult,
                    )
                    nc.sync.dma_start(
                        out=o[dc * P : (dc + 1) * P, b0 : b0 + BT], in_=ct[:]
                    )

    nc.compile()
    return nc


def _get_nc():
    if "nc" not in _NC_CACHE:
        _NC_CACHE["nc"] = _build_nc()
    return _NC_CACHE["nc"]


def _prep_in_maps(inputs):
    x = np.asarray(inputs["x"], np.float32)
    Wr = np.asarray(inputs["Wr"], np.float32)
    br = np.asarray(inputs["br"], np.float32)
    W1 = np.asarray(inputs["W1"], np.float32)
    b1 = np.asarray(inputs["b1"], np.float32)
    W2 = np.asarray(inputs["W2"], np.float32)
    b2 = np.asarray(inputs["b2"], np.float32)

    # x [B, D] -> [p, dc2, bt, i, n] with b = bt*BT+n, d = dc2*256+i*128+p
    x8 = (x * SX).astype(_E4M3)
    xt8 = np.ascontiguousarray(
        x8.reshape(NB, BT, DC2, 2, P).transpose(4, 2, 0, 3, 1).reshape(P, -1))

    # Wr [D, E] -> [p, dc2, i, e]
    wr8 = np.ascontiguousarray(
        (Wr * SWR).astype(_E4M3)
        .reshape(DC2, 2, P, E).transpose(2, 0, 1, 3).reshape(P, -1))

    br_c = np.ascontiguousarray(br.reshape(E, 1))
    ones8_c = np.ones((E, 1), _BF16)
    ones18_c = np.ones((1, E), _BF16)

    in_maps = []
    for c in range(N_CORES):
        # W1[c] [D, U] -> [p, dc2, uc, i, m] with d = dc2*256+i*128+p,
        # u = uc*128+m
        w1p = np.ascontiguousarray(
            (W1[c] * SW1).astype(_E4M3)
            .reshape(DC2, 2, P, UC, P).transpose(2, 0, 3, 1, 4).reshape(P, -1))
        # W2[c] [U, D] -> [p, g, dc, i, m] with u = g*256+i*128+p,
        # d = dc*128+m
        w2p = np.ascontiguousarray(
            (W2[c] * SW2).astype(_E4M3)
            .reshape(UG, 2, P, DC, P).transpose(2, 0, 3, 1, 4).reshape(P, -1))
        sel = np.zeros((E, P), np.float32)
        sel[c, :] = 1.0 / SW2
        in_maps.append({
            "xt": xt8,
            "w1": w1p,
            "w2": w2p,
            "wr": wr8,
            "br": br_c,
            "b1": np.ascontiguousarray(b1[c].reshape(UC, P).T),
            "b2": np.ascontiguousarray((b2[c] * SW2).reshape(DC, P).T),
            "selb": sel.astype(_BF16),
            "ones8": ones8_c,
            "ones18": ones18_c,
        })
    return in_maps


def kernel(**inputs):
    from concourse.bass_utils import run_bass_kernel_spmd

    global LAST_RESULTS

    in_maps = _prep_in_maps(inputs)
    nc = _get_nc()
    want_trace = bool(int(os.environ.get("KERNEL_TRACE", "0")))
    if not want_trace:
        # the NTFF-trace path needs antenv.axon_hooks, which this container
        # lacks; make sure a stray BASS_TRACE env can't route us into it
        os.environ["BASS_NEVER_TRACE"] = "1"
    res = run_bass_kernel_spmd(
        nc, in_maps, core_ids=list(range(N_CORES)), trace=want_trace,
    )
    LAST_RESULTS = res

    # host: 8-way partial-sum reduction + residual + transpose back
    acc = res.results[0]["o"].astype(np.float32, copy=True)
    for c in range(1, N_CORES):
        acc += res.results[c]["o"]
    out = acc.T + np.asarray(inputs["x"], np.float32)
    return np.ascontiguousarray(out)
